# revision 40
# baseline (speedup 1.0000x reference)
"""Trainium2 Bass kernel for nn_BlockWavelet_Down (B=64, Cin=64, Cout=128, L=4096).

Pure data parallel over 8 NeuronCores: 8 samples per core.

Per-core pipeline (channels on 128 SBUF partitions, length on free dim):
  conv1 (grouped 64->128, k=7)  : TensorE f32r matmuls, 2 taps packed per pass
                                  via a duplicated+shifted copy of x in K
  BN + exact GELU               : one ACT op (per-partition scale/bias)
  leaky_relu                    : ACT Lrelu
  Haar DWT                      : DVE tensor_tensor on stride-2 APs
  conv2L/H (depthwise k=7/k=3)  : TensorE diagonal f32r matmuls (1/sqrt2 factors
                                  folded into the weights)
  Haar IDWT + skip + leaky      : DVE tensor_tensor / scalar_tensor_tensor
"""

import os
import sys

for _p in ("/opt/trn_rl_repo",):
    if _p not in sys.path and os.path.isdir(_p):
        sys.path.insert(0, _p)

import numpy as np

import concourse.bass as bass
import concourse.bacc as bacc
import concourse.mybir as mybir
from concourse.tile import TileContext
from concourse.bass_utils import run_bass_kernel_spmd

F32 = mybir.dt.float32
F32R = mybir.dt.float32r
AF = mybir.ActivationFunctionType
OP = mybir.AluOpType

NCORES = 8
S = 8          # samples per core
CIN = 64
C = 128
L = 4096
L2 = L // 2
CH = 1024      # conv1/output columns per chunk
NCH = L // CH
T = CH // 2    # conv2 columns per chunk
EPS = 1e-5

# conv1 tap pairs: pass j covers taps (k, k+1) for k in K1BASE (tap offsets in
# [-3, 3]; last pass covers tap 3 only, rows 64.. of its lhsT are zero).
K1BASE = (-3, -1, 1, 3)


def build_program(gelu_func=AF.Gelu, leaky_on_act=None, repeat=1,
                  leaky_func=AF.Prelu, staggered=False):
    # Prelu (parametric_relu) lives in the same ACT table set as Gelu
    # ("gelu_and_others"), so using it for leaky_relu avoids the ~2.7us
    # table reload that alternating Gelu/Lrelu would trigger.
    if leaky_on_act is None:
        leaky_on_act = os.environ.get("KERNEL_LEAKY_ACT", "1") == "1"
    """Builds the SPMD Bass program. gelu_func/leaky_on_act can be swapped for
    CoreSim (which does not implement Gelu/Lrelu). repeat>1 wraps the body in
    a hardware loop (benchmark amplification only)."""
    nc = bacc.Bacc()

    x_in = nc.declare_dram_parameter("x", [S, CIN, L], F32R, isOutput=False)
    w_c1 = nc.declare_dram_parameter("w_c1", [4, 128, 128], F32R, isOutput=False)
    w_c2 = nc.declare_dram_parameter("w_c2", [12, 128, 128], F32R, isOutput=False)
    vecs = nc.declare_dram_parameter("vecs", [128, 4], F32, isOutput=False)
    y_out = nc.declare_dram_parameter("y", [S, C, L], F32, isOutput=True)

    with TileContext(nc) as tc:
        with (
            tc.tile_pool(name="wpool", bufs=1) as wpool,
            tc.tile_pool(name="px", bufs=2) as px,
            tc.tile_pool(name="psd", bufs=3) as psd,
            tc.tile_pool(name="ph", bufs=4) as ph,
            tc.tile_pool(name="pf", bufs=4) as pf,
            tc.tile_pool(name="psum_h", bufs=2, space="PSUM") as psum_h,
            tc.tile_pool(name="psum_a", bufs=2, space="PSUM") as psum_a,
            tc.tile_pool(name="psum_d", bufs=2, space="PSUM") as psum_d,
        ):
            wc1 = wpool.tile([128, 4 * 128], F32R, tag="wc1")
            for j in range(4):
                nc.sync.dma_start(out=wc1[:, j * 128:(j + 1) * 128], in_=w_c1[j])
            wc2 = wpool.tile([128, 12 * 128], F32R, tag="wc2")
            for j in range(12):
                nc.sync.dma_start(out=wc2[:, j * 128:(j + 1) * 128], in_=w_c2[j])
            vt = wpool.tile([128, 4], F32, tag="vecs")
            nc.sync.dma_start(out=vt[:], in_=vecs[:])
            v_scale = vt[:, 0:1]
            v_bias = vt[:, 1:2]
            v_skip = vt[:, 2:3]

            import contextlib
            loop_cm = (tc.For_i(0, repeat, 1,
                                hint_engines=(mybir.EngineType.PE,),
                                staggered_reset=staggered)
                       if repeat > 1 else contextlib.nullcontext())
            with loop_cm:
                _build_body(nc, tc, px, psd, ph, pf,
                            psum_h, psum_a, psum_d,
                            x_in, y_out, wc1, wc2,
                            v_scale, v_bias, v_skip,
                            gelu_func, leaky_on_act, leaky_func)

    nc.finalize()
    return nc


def _build_body(nc, tc, px, psd, ph, pf, psum_h, psum_a, psum_d,
                x_in, y_out, wc1, wc2, v_scale, v_bias, v_skip,
                gelu_func, leaky_on_act, leaky_func):
    if True:
        if True:
            for s in range(S):
                # xs2: partitions 0-63 hold x (col c = x[c-3]); partitions
                # 64-127 hold x shifted one column (col c = x[c-2]).
                xs2 = px.tile([128, L + 8], F32R, tag="xs2")
                nc.gpsimd.memset(xs2[0:64, 0:3].bitcast(F32), 0.0)
                nc.gpsimd.memset(xs2[0:64, L + 3:L + 8].bitcast(F32), 0.0)
                nc.gpsimd.memset(xs2[64:128, 0:2].bitcast(F32), 0.0)
                nc.gpsimd.memset(xs2[64:128, L + 2:L + 8].bitcast(F32), 0.0)
                nc.gpsimd.dma_start(out=xs2[0:64, 3:L + 3], in_=x_in[s])
                nc.gpsimd.dma_start(out=xs2[64:128, 2:L + 2],
                                    in_=xs2[0:64, 3:L + 3])

                # s/d hold the haar sum/diff bands, padded 3 cols each side.
                sfull = psd.tile([128, L2 + 6], F32R, tag="s")
                dfull = psd.tile([128, L2 + 6], F32R, tag="d")
                nc.gpsimd.memset(sfull[:, 0:3].bitcast(F32), 0.0)
                nc.gpsimd.memset(sfull[:, L2 + 3:L2 + 6].bitcast(F32), 0.0)
                nc.gpsimd.memset(dfull[:, 0:3].bitcast(F32), 0.0)
                nc.gpsimd.memset(dfull[:, L2 + 3:L2 + 6].bitcast(F32), 0.0)

                for ci in range(NCH):
                    l0 = ci * CH
                    hp = psum_h.tile([128, CH], F32, tag="hp")
                    for half in range(2):
                        o0 = half * 512
                        for j, k in enumerate(K1BASE):
                            nc.tensor.matmul(
                                hp[:, o0:o0 + 512],
                                wc1[:, j * 128:(j + 1) * 128],
                                xs2[:, l0 + 3 + k + o0:l0 + 3 + k + o0 + 512],
                                start=(j == 0),
                                stop=(j == 3),
                            )
                    # h = gelu(scale * conv1 + bias)  (BN folded in)
                    h = ph.tile([128, CH], F32, tag="h")
                    nc.scalar.activation(h[:], hp[:], gelu_func,
                                         bias=v_bias, scale=v_scale)
                    hl = ph.tile([128, CH], F32, tag="hl")
                    if leaky_on_act:
                        nc.scalar.activation(hl[:], h[:], leaky_func, alpha=0.01)
                    else:
                        nc.vector.scalar_tensor_tensor(
                            hl[:], h[:], 0.01, h[:], OP.mult, OP.max)
                    # haar DWT (scale folded into conv2 weights)
                    t0 = l0 // 2
                    nc.vector.tensor_tensor(
                        sfull[:, 3 + t0:3 + t0 + T],
                        hl[:, 0:CH:2], hl[:, 1:CH:2], OP.add)
                    nc.vector.tensor_tensor(
                        dfull[:, 3 + t0:3 + t0 + T],
                        hl[:, 0:CH:2], hl[:, 1:CH:2], OP.subtract)

                for ci in range(NCH):
                    t0 = ci * T
                    l0 = ci * CH
                    pa = psum_a.tile([128, T], F32, tag="pa")
                    pd = psum_d.tile([128, T], F32, tag="pd")
                    for j in range(7):
                        nc.tensor.matmul(
                            pa[:],
                            wc2[:, j * 128:(j + 1) * 128],
                            sfull[:, t0 + j:t0 + j + T],
                            start=(j == 0), stop=False,
                        )
                    for m in range(3):
                        nc.tensor.matmul(
                            pd[:],
                            wc2[:, (7 + m) * 128:(8 + m) * 128],
                            dfull[:, t0 + 2 + m:t0 + 2 + m + T],
                            start=(m == 0), stop=False,
                        )
                    # skip (1x1 grouped conv) folded into the IDWT psums:
                    # A += (sk_e + sk_o)/2, D += (sk_e - sk_o)/2 via one
                    # stride-2 rhs over the double-copy layout (lower
                    # partitions see x_even, upper see x_odd).
                    rhs_sk = xs2[:, 3 + 2 * t0: 3 + 2 * t0 + 2 * T: 2]
                    nc.tensor.matmul(pa[:], wc2[:, 10 * 128:11 * 128],
                                     rhs_sk, start=False, stop=True)
                    nc.tensor.matmul(pd[:], wc2[:, 11 * 128:12 * 128],
                                     rhs_sk, start=False, stop=True)
                    # drain A to SBUF (HW: only one TT operand may be PSUM)
                    asb = pf.tile([128, T], F32, tag="asb")
                    nc.vector.tensor_copy(asb[:], pa[:])
                    # haar IDWT: interleave (A+D, A-D) = rec + skip
                    f = pf.tile([128, CH], F32, tag="f")
                    nc.vector.tensor_tensor(f[:, 0:CH:2], asb[:], pd[:], OP.add)
                    nc.vector.tensor_tensor(f[:, 1:CH:2], asb[:], pd[:], OP.subtract)
                    # final leaky_relu
                    o = pf.tile([128, CH], F32, tag="o")
                    if leaky_on_act:
                        nc.scalar.activation(o[:], f[:], leaky_func, alpha=0.01)
                    else:
                        nc.vector.scalar_tensor_tensor(
                            o[:], f[:], 0.01, f[:], OP.mult, OP.max)
                    # un-permute channels: partition q -> channel 2q,
                    # partition 64+q -> channel 2q+1
                    nc.sync.dma_start(out=y_out[s, 0:C:2, l0:l0 + CH],
                                      in_=o[0:64, :])
                    nc.sync.dma_start(out=y_out[s, 1:C:2, l0:l0 + CH],
                                      in_=o[64:128, :])


def prepare_weights(w1, g1, b1, m1, v1, w2L, w2H, w_skip):
    """Host-side packing of all weight/BN tensors into the DRAM params."""
    w1 = np.asarray(w1, np.float32).reshape(C, 7)
    w2L = np.asarray(w2L, np.float32).reshape(C, 7)
    w2H = np.asarray(w2H, np.float32).reshape(C, 3)
    w_skip = np.asarray(w_skip, np.float32).reshape(C)
    g1 = np.asarray(g1, np.float32)
    b1 = np.asarray(b1, np.float32)
    m1 = np.asarray(m1, np.float32)
    v1 = np.asarray(v1, np.float32)

    # Channel permutation: SBUF partition p holds output channel phi(p).
    phi = np.array([2 * (p % 64) + p // 64 for p in range(128)])

    # conv1 lhsT per pass: [K=128, M=128]; rows 0-63 tap k, rows 64-127 tap k+1
    w_c1 = np.zeros((4, 128, 128), np.float32)
    for j, k in enumerate(K1BASE):
        for p in range(128):
            c = phi[p]
            i = p % 64  # = c // 2
            w_c1[j, i, p] = w1[c, k + 3]
            if k + 1 <= 3:  # last pass has no second tap
                w_c1[j, 64 + i, p] = w1[c, k + 4]

    # conv2 diagonal lhsT; both 1/sqrt2 haar factors folded in => /2
    w_c2 = np.zeros((12, 128, 128), np.float32)
    for j in range(7):
        np.fill_diagonal(w_c2[j], w2L[phi, j] * 0.5)
    for m in range(3):
        np.fill_diagonal(w_c2[7 + m], w2H[phi, m] * 0.5)
    # skip folded into the IDWT psums: A += (sk_e+sk_o)/2, D += (sk_e-sk_o)/2
    for p in range(128):
        i = p % 64
        w = w_skip[phi[p]] * 0.5
        w_c2[10, i, p] = w        # x_even rows
        w_c2[10, 64 + i, p] = w   # x_odd rows
        w_c2[11, i, p] = w
        w_c2[11, 64 + i, p] = -w

    inv = g1 / np.sqrt(v1 + EPS)
    bias = b1 - m1 * inv
    vecs = np.zeros((128, 4), np.float32)
    vecs[:, 0] = inv[phi]            # ACT scale
    vecs[:, 1] = bias[phi]           # ACT bias
    vecs[:, 2] = w_skip[phi]         # skip scale per out-channel
    return w_c1, w_c2, vecs


_NC_CACHE = {}


def _get_program():
    key = "hw"
    if key not in _NC_CACHE:
        _NC_CACHE[key] = build_program()
    return _NC_CACHE[key]


def kernel(x, w1, g1, b1, m1, v1, w2L, w2H, w_skip):
    x = np.ascontiguousarray(np.asarray(x, np.float32))
    assert x.shape == (NCORES * S, CIN, L), x.shape

    w_c1, w_c2, vecs = prepare_weights(w1, g1, b1, m1, v1, w2L, w2H, w_skip)

    nc = _get_program()
    in_maps = []
    for i in range(NCORES):
        in_maps.append({
            "x": np.ascontiguousarray(x[i * S:(i + 1) * S]),
            "w_c1": w_c1,
            "w_c2": w_c2,
            "vecs": vecs,
        })
    res = run_bass_kernel_spmd(nc, in_maps, list(range(NCORES))).results
    out = np.concatenate([res[i]["y"] for i in range(NCORES)], axis=0)
    return out


# revision 43
# speedup vs baseline: 1.0313x; 1.0313x over previous
"""Trainium2 Bass kernel for nn_BlockWavelet_Down (B=64, Cin=64, Cout=128, L=4096).

Pure data parallel over 8 NeuronCores: 8 samples per core.

Per-core pipeline (channels on 128 SBUF partitions, length on free dim;
channels stored in permuted order phi(p) = 2*(p%64) + p//64 so all partition
writes are contiguous blocks):
  conv1 (grouped 64->128, k=7)  : TensorE float32r matmuls, 2 taps packed per
                                  pass via a duplicated+shifted copy of x in K
                                  (4 passes instead of 7)
  BN + exact GELU               : one ACT op (BN folded into scale/bias)
  leaky_relu (x2)               : ACT Prelu (parametric_relu shares the
                                  "gelu_and_others" table set -> no ACT
                                  table reloads; Lrelu would thrash ~2.7us/op)
  Haar DWT                      : DVE tensor_tensor on stride-2 APs
  conv2L/H (depthwise k=7/k=3)  : TensorE diagonal f32r matmuls; both 1/sqrt2
                                  haar factors folded into the weights
  skip (1x1 grouped conv)       : folded into the conv2 PSUMs as 2 extra
                                  matmuls: A += (sk_e+sk_o)/2, D += (sk_e-sk_o)/2
                                  over a stride-2 rhs on the double-copy layout
  Haar IDWT (+skip) + leaky     : DVE tensor_tensor (PSUM) + ACT Prelu
DMA queues: x load + shift copy on GPSIMD SWDGE, y store + weights on SP
HWDGE - every DMA occupies its issuing engine-queue for the transfer, so
traffic is spread to keep all five engines ~balanced (PE is the bottleneck).
"""

import os
import sys

for _p in ("/opt/trn_rl_repo",):
    if _p not in sys.path and os.path.isdir(_p):
        sys.path.insert(0, _p)

import numpy as np

import concourse.bacc as bacc
import concourse.mybir as mybir
from concourse.tile import TileContext
from concourse.bass_utils import run_bass_kernel_spmd

F32 = mybir.dt.float32
F32R = mybir.dt.float32r
AF = mybir.ActivationFunctionType
OP = mybir.AluOpType

NCORES = 8
S = 8          # samples per core
CIN = 64
C = 128
L = 4096
L2 = L // 2
CH = 1024      # conv1/output columns per chunk
NCH = L // CH
T = CH // 2    # conv2 columns per chunk
EPS = 1e-5

# conv1 tap pairs: pass j covers taps (k, k+1) for k in K1BASE (tap offsets in
# [-3, 3]; last pass covers tap 3 only, rows 64.. of its lhsT are zero).
K1BASE = (-3, -1, 1, 3)


def build_program(gelu_func=AF.Gelu, leaky_on_act=None, repeat=1,
                  leaky_func=AF.Prelu, staggered=False, chunk_load=True):
    # Prelu (parametric_relu) lives in the same ACT table set as Gelu
    # ("gelu_and_others"), so using it for leaky_relu avoids the ~2.7us
    # table reload that alternating Gelu/Lrelu would trigger.
    if leaky_on_act is None:
        leaky_on_act = os.environ.get("KERNEL_LEAKY_ACT", "1") == "1"
    """Builds the SPMD Bass program. gelu_func/leaky_on_act can be swapped for
    CoreSim (which does not implement Gelu/Lrelu). repeat>1 wraps the body in
    a hardware loop (benchmark amplification only)."""
    nc = bacc.Bacc()

    x_in = nc.declare_dram_parameter("x", [S, CIN, L], F32R, isOutput=False)
    w_c1 = nc.declare_dram_parameter("w_c1", [4, 128, 128], F32R, isOutput=False)
    w_c2 = nc.declare_dram_parameter("w_c2", [12, 128, 128], F32R, isOutput=False)
    vecs = nc.declare_dram_parameter("vecs", [128, 4], F32, isOutput=False)
    y_out = nc.declare_dram_parameter("y", [S, C, L], F32, isOutput=True)

    with TileContext(nc) as tc:
        with (
            tc.tile_pool(name="wpool", bufs=1) as wpool,
            tc.tile_pool(name="px", bufs=2) as px,
            tc.tile_pool(name="psd", bufs=3) as psd,
            tc.tile_pool(name="ph", bufs=4) as ph,
            tc.tile_pool(name="pf", bufs=4) as pf,
            tc.tile_pool(name="psum_h", bufs=2, space="PSUM") as psum_h,
            tc.tile_pool(name="psum_a", bufs=2, space="PSUM") as psum_a,
            tc.tile_pool(name="psum_d", bufs=2, space="PSUM") as psum_d,
        ):
            wc1 = wpool.tile([128, 4 * 128], F32R, tag="wc1")
            for j in range(4):
                nc.sync.dma_start(out=wc1[:, j * 128:(j + 1) * 128], in_=w_c1[j])
            wc2 = wpool.tile([128, 12 * 128], F32R, tag="wc2")
            for j in range(12):
                nc.sync.dma_start(out=wc2[:, j * 128:(j + 1) * 128], in_=w_c2[j])
            vt = wpool.tile([128, 4], F32, tag="vecs")
            nc.sync.dma_start(out=vt[:], in_=vecs[:])
            v_scale = vt[:, 0:1]
            v_bias = vt[:, 1:2]

            import contextlib
            loop_cm = (tc.For_i(0, repeat, 1,
                                hint_engines=(mybir.EngineType.PE,),
                                staggered_reset=staggered)
                       if repeat > 1 else contextlib.nullcontext())
            with loop_cm:
                _build_body(nc, tc, px, psd, ph, pf,
                            psum_h, psum_a, psum_d,
                            x_in, y_out, wc1, wc2,
                            v_scale, v_bias,
                            gelu_func, leaky_on_act, leaky_func, chunk_load)

    nc.finalize()
    return nc


def _build_body(nc, tc, px, psd, ph, pf, psum_h, psum_a, psum_d,
                x_in, y_out, wc1, wc2, v_scale, v_bias,
                gelu_func, leaky_on_act, leaky_func, chunk_load=False):
    if True:
        if True:
            for s in range(S):
                # xs2: partitions 0-63 hold x (col c = x[c-3]); partitions
                # 64-127 hold x shifted one column (col c = x[c-2]).
                xs2 = px.tile([128, L + 8], F32R, tag="xs2")
                nc.gpsimd.memset(xs2[0:64, 0:3].bitcast(F32), 0.0)
                nc.gpsimd.memset(xs2[0:64, L + 3:L + 8].bitcast(F32), 0.0)
                nc.gpsimd.memset(xs2[64:128, 0:2].bitcast(F32), 0.0)
                nc.gpsimd.memset(xs2[64:128, L + 2:L + 8].bitcast(F32), 0.0)
                if chunk_load:
                    for ci in range(2):
                        l0 = ci * (L // 2)
                        w = L // 2
                        nc.gpsimd.dma_start(
                            out=xs2[0:64, 3 + l0:3 + l0 + w],
                            in_=x_in[s, :, l0:l0 + w])
                        nc.gpsimd.dma_start(
                            out=xs2[64:128, 2 + l0:2 + l0 + w],
                            in_=xs2[0:64, 3 + l0:3 + l0 + w])
                else:
                    nc.gpsimd.dma_start(out=xs2[0:64, 3:L + 3], in_=x_in[s])
                    nc.gpsimd.dma_start(out=xs2[64:128, 2:L + 2],
                                        in_=xs2[0:64, 3:L + 3])

                # s/d hold the haar sum/diff bands, padded 3 cols each side.
                sfull = psd.tile([128, L2 + 6], F32R, tag="s")
                dfull = psd.tile([128, L2 + 6], F32R, tag="d")
                nc.gpsimd.memset(sfull[:, 0:3].bitcast(F32), 0.0)
                nc.gpsimd.memset(sfull[:, L2 + 3:L2 + 6].bitcast(F32), 0.0)
                nc.gpsimd.memset(dfull[:, 0:3].bitcast(F32), 0.0)
                nc.gpsimd.memset(dfull[:, L2 + 3:L2 + 6].bitcast(F32), 0.0)

                for ci in range(NCH):
                    l0 = ci * CH
                    hp = psum_h.tile([128, CH], F32, tag="hp")
                    for half in range(2):
                        o0 = half * 512
                        for j, k in enumerate(K1BASE):
                            nc.tensor.matmul(
                                hp[:, o0:o0 + 512],
                                wc1[:, j * 128:(j + 1) * 128],
                                xs2[:, l0 + 3 + k + o0:l0 + 3 + k + o0 + 512],
                                start=(j == 0),
                                stop=(j == 3),
                            )
                    # h = gelu(scale * conv1 + bias)  (BN folded in)
                    h = ph.tile([128, CH], F32, tag="h")
                    nc.scalar.activation(h[:], hp[:], gelu_func,
                                         bias=v_bias, scale=v_scale)
                    hl = ph.tile([128, CH], F32, tag="hl")
                    if leaky_on_act:
                        nc.scalar.activation(hl[:], h[:], leaky_func, alpha=0.01)
                    else:
                        nc.vector.scalar_tensor_tensor(
                            hl[:], h[:], 0.01, h[:], OP.mult, OP.max)
                    # haar DWT (scale folded into conv2 weights)
                    t0 = l0 // 2
                    nc.vector.tensor_tensor(
                        sfull[:, 3 + t0:3 + t0 + T],
                        hl[:, 0:CH:2], hl[:, 1:CH:2], OP.add)
                    nc.vector.tensor_tensor(
                        dfull[:, 3 + t0:3 + t0 + T],
                        hl[:, 0:CH:2], hl[:, 1:CH:2], OP.subtract)

                for ci in range(NCH):
                    t0 = ci * T
                    l0 = ci * CH
                    pa = psum_a.tile([128, T], F32, tag="pa")
                    pd = psum_d.tile([128, T], F32, tag="pd")
                    for j in range(7):
                        nc.tensor.matmul(
                            pa[:],
                            wc2[:, j * 128:(j + 1) * 128],
                            sfull[:, t0 + j:t0 + j + T],
                            start=(j == 0), stop=False,
                        )
                    for m in range(3):
                        nc.tensor.matmul(
                            pd[:],
                            wc2[:, (7 + m) * 128:(8 + m) * 128],
                            dfull[:, t0 + 2 + m:t0 + 2 + m + T],
                            start=(m == 0), stop=False,
                        )
                    # skip (1x1 grouped conv) folded into the IDWT psums:
                    # A += (sk_e + sk_o)/2, D += (sk_e - sk_o)/2 via one
                    # stride-2 rhs over the double-copy layout (lower
                    # partitions see x_even, upper see x_odd).
                    rhs_sk = xs2[:, 3 + 2 * t0: 3 + 2 * t0 + 2 * T: 2]
                    nc.tensor.matmul(pa[:], wc2[:, 10 * 128:11 * 128],
                                     rhs_sk, start=False, stop=True)
                    nc.tensor.matmul(pd[:], wc2[:, 11 * 128:12 * 128],
                                     rhs_sk, start=False, stop=True)
                    # drain A to SBUF (HW: only one TT operand may be PSUM)
                    asb = pf.tile([128, T], F32, tag="asb")
                    nc.vector.tensor_copy(asb[:], pa[:])
                    # haar IDWT: interleave (A+D, A-D) = rec + skip
                    f = pf.tile([128, CH], F32, tag="f")
                    nc.vector.tensor_tensor(f[:, 0:CH:2], asb[:], pd[:], OP.add)
                    nc.vector.tensor_tensor(f[:, 1:CH:2], asb[:], pd[:], OP.subtract)
                    # final leaky_relu
                    o = pf.tile([128, CH], F32, tag="o")
                    if leaky_on_act:
                        nc.scalar.activation(o[:], f[:], leaky_func, alpha=0.01)
                    else:
                        nc.vector.scalar_tensor_tensor(
                            o[:], f[:], 0.01, f[:], OP.mult, OP.max)
                    # un-permute channels: partition q -> channel 2q,
                    # partition 64+q -> channel 2q+1
                    nc.sync.dma_start(out=y_out[s, 0:C:2, l0:l0 + CH],
                                      in_=o[0:64, :])
                    nc.sync.dma_start(out=y_out[s, 1:C:2, l0:l0 + CH],
                                      in_=o[64:128, :])


def prepare_weights(w1, g1, b1, m1, v1, w2L, w2H, w_skip):
    """Host-side packing of all weight/BN tensors into the DRAM params."""
    w1 = np.asarray(w1, np.float32).reshape(C, 7)
    w2L = np.asarray(w2L, np.float32).reshape(C, 7)
    w2H = np.asarray(w2H, np.float32).reshape(C, 3)
    w_skip = np.asarray(w_skip, np.float32).reshape(C)
    g1 = np.asarray(g1, np.float32)
    b1 = np.asarray(b1, np.float32)
    m1 = np.asarray(m1, np.float32)
    v1 = np.asarray(v1, np.float32)

    # Channel permutation: SBUF partition p holds output channel phi(p).
    phi = np.array([2 * (p % 64) + p // 64 for p in range(128)])

    # conv1 lhsT per pass: [K=128, M=128]; rows 0-63 tap k, rows 64-127 tap k+1
    w_c1 = np.zeros((4, 128, 128), np.float32)
    for j, k in enumerate(K1BASE):
        for p in range(128):
            c = phi[p]
            i = p % 64  # = c // 2
            w_c1[j, i, p] = w1[c, k + 3]
            if k + 1 <= 3:  # last pass has no second tap
                w_c1[j, 64 + i, p] = w1[c, k + 4]

    # conv2 diagonal lhsT; both 1/sqrt2 haar factors folded in => /2
    w_c2 = np.zeros((12, 128, 128), np.float32)
    for j in range(7):
        np.fill_diagonal(w_c2[j], w2L[phi, j] * 0.5)
    for m in range(3):
        np.fill_diagonal(w_c2[7 + m], w2H[phi, m] * 0.5)
    # skip folded into the IDWT psums: A += (sk_e+sk_o)/2, D += (sk_e-sk_o)/2
    for p in range(128):
        i = p % 64
        w = w_skip[phi[p]] * 0.5
        w_c2[10, i, p] = w        # x_even rows
        w_c2[10, 64 + i, p] = w   # x_odd rows
        w_c2[11, i, p] = w
        w_c2[11, 64 + i, p] = -w

    inv = g1 / np.sqrt(v1 + EPS)
    bias = b1 - m1 * inv
    vecs = np.zeros((128, 4), np.float32)
    vecs[:, 0] = inv[phi]            # ACT scale
    vecs[:, 1] = bias[phi]           # ACT bias
    vecs[:, 2] = w_skip[phi]         # skip scale per out-channel
    return w_c1, w_c2, vecs


_NC_CACHE = {}


def _get_program():
    key = "hw"
    if key not in _NC_CACHE:
        _NC_CACHE[key] = build_program()
    return _NC_CACHE[key]


def kernel(x, w1, g1, b1, m1, v1, w2L, w2H, w_skip):
    x = np.ascontiguousarray(np.asarray(x, np.float32))
    assert x.shape == (NCORES * S, CIN, L), x.shape

    w_c1, w_c2, vecs = prepare_weights(w1, g1, b1, m1, v1, w2L, w2H, w_skip)

    nc = _get_program()
    in_maps = []
    for i in range(NCORES):
        in_maps.append({
            "x": np.ascontiguousarray(x[i * S:(i + 1) * S]),
            "w_c1": w_c1,
            "w_c2": w_c2,
            "vecs": vecs,
        })
    res = run_bass_kernel_spmd(nc, in_maps, list(range(NCORES))).results
    out = np.concatenate([res[i]["y"] for i in range(NCORES)], axis=0)
    return out


# revision 47
# speedup vs baseline: 1.0924x; 1.0592x over previous
"""Trainium2 Bass kernel for nn_BlockWavelet_Down (B=64, Cin=64, Cout=128, L=4096).

Pure data parallel over 8 NeuronCores: 8 samples per core.

Per-core pipeline (channels on 128 SBUF partitions, length on free dim;
channels stored in permuted order phi(p) = 2*(p%64) + p//64 so all partition
writes are contiguous blocks):
  conv1 (grouped 64->128, k=7)  : TensorE float32r matmuls, 2 taps packed per
                                  pass via a duplicated+shifted copy of x in K
                                  (4 passes instead of 7)
  BN + exact GELU               : one ACT op (BN folded into scale/bias)
  leaky_relu (x2)               : ACT Prelu (parametric_relu shares the
                                  "gelu_and_others" table set -> no ACT
                                  table reloads; Lrelu would thrash ~2.7us/op)
  Haar DWT                      : DVE tensor_tensor on stride-2 APs
  conv2L/H (depthwise k=7/k=3)  : TensorE diagonal f32r matmuls; both 1/sqrt2
                                  haar factors folded into the weights
  skip (1x1 grouped conv)       : folded into the conv2 PSUMs as 2 extra
                                  matmuls: A += (sk_e+sk_o)/2, D += (sk_e-sk_o)/2
                                  over a stride-2 rhs on the double-copy layout
  Haar IDWT (+skip) + leaky     : DVE tensor_tensor (PSUM) + ACT Prelu
DMA queues: x load + shift copy on GPSIMD SWDGE, y store + weights on SP
HWDGE - every DMA occupies its issuing engine-queue for the transfer, so
traffic is spread to keep all five engines ~balanced (PE is the bottleneck).
"""

import os
import sys

for _p in ("/opt/trn_rl_repo",):
    if _p not in sys.path and os.path.isdir(_p):
        sys.path.insert(0, _p)

import numpy as np

import concourse.bacc as bacc
import concourse.mybir as mybir
from concourse.tile import TileContext
from concourse.bass_utils import run_bass_kernel_spmd

F32 = mybir.dt.float32
F32R = mybir.dt.float32r
AF = mybir.ActivationFunctionType
OP = mybir.AluOpType

NCORES = 8
S = 8          # samples per core
CIN = 64
C = 128
L = 4096
L2 = L // 2
CH = 1024      # conv1/output columns per chunk
NCH = L // CH
T = CH // 2    # conv2 columns per chunk
EPS = 1e-5

# conv1 tap pairs: pass j covers taps (k, k+1) for k in K1BASE (tap offsets in
# [-3, 3]; last pass covers tap 3 only, rows 64.. of its lhsT are zero).
K1BASE = (-3, -1, 1, 3)


def build_program(gelu_func=AF.Gelu, leaky_on_act=None, repeat=1,
                  leaky_func=AF.Prelu, staggered=False, chunk_load=True):
    # Prelu (parametric_relu) lives in the same ACT table set as Gelu
    # ("gelu_and_others"), so using it for leaky_relu avoids the ~2.7us
    # table reload that alternating Gelu/Lrelu would trigger.
    if leaky_on_act is None:
        leaky_on_act = os.environ.get("KERNEL_LEAKY_ACT", "1") == "1"
    """Builds the SPMD Bass program. gelu_func/leaky_on_act can be swapped for
    CoreSim (which does not implement Gelu/Lrelu). repeat>1 wraps the body in
    a hardware loop (benchmark amplification only)."""
    nc = bacc.Bacc()

    x_in = nc.declare_dram_parameter("x", [S, CIN, L], F32R, isOutput=False)
    w_c1 = nc.declare_dram_parameter("w_c1", [4, 128, 128], F32R, isOutput=False)
    w_c2 = nc.declare_dram_parameter("w_c2", [12, 128, 128], F32R, isOutput=False)
    vecs = nc.declare_dram_parameter("vecs", [128, 4], F32, isOutput=False)
    y_out = nc.declare_dram_parameter("y", [S, C, L], F32, isOutput=True)

    with TileContext(nc) as tc:
        with (
            tc.tile_pool(name="wpool", bufs=1) as wpool,
            tc.tile_pool(name="px", bufs=2) as px,
            tc.tile_pool(name="psd", bufs=3) as psd,
            tc.tile_pool(name="ph", bufs=4) as ph,
            tc.tile_pool(name="pf", bufs=4) as pf,
            tc.tile_pool(name="psum_h", bufs=2, space="PSUM") as psum_h,
            tc.tile_pool(name="psum_a", bufs=2, space="PSUM") as psum_a,
            tc.tile_pool(name="psum_d", bufs=2, space="PSUM") as psum_d,
        ):
            wc1 = wpool.tile([128, 4 * 128], F32R, tag="wc1")
            for j in range(4):
                nc.sync.dma_start(out=wc1[:, j * 128:(j + 1) * 128], in_=w_c1[j])
            wc2 = wpool.tile([128, 12 * 128], F32R, tag="wc2")
            for j in range(12):
                nc.sync.dma_start(out=wc2[:, j * 128:(j + 1) * 128], in_=w_c2[j])
            vt = wpool.tile([128, 4], F32, tag="vecs")
            nc.sync.dma_start(out=vt[:], in_=vecs[:])
            v_scale = vt[:, 0:1]
            v_bias = vt[:, 1:2]

            import contextlib
            loop_cm = (tc.For_i(0, repeat, 1,
                                hint_engines=(mybir.EngineType.PE,),
                                staggered_reset=staggered)
                       if repeat > 1 else contextlib.nullcontext())
            with loop_cm:
                _build_body(nc, tc, px, psd, ph, pf,
                            psum_h, psum_a, psum_d,
                            x_in, y_out, wc1, wc2,
                            v_scale, v_bias,
                            gelu_func, leaky_on_act, leaky_func, chunk_load)

    nc.finalize()
    return nc


def _build_body(nc, tc, px, psd, ph, pf, psum_h, psum_a, psum_d,
                x_in, y_out, wc1, wc2, v_scale, v_bias,
                gelu_func, leaky_on_act, leaky_func, chunk_load=False):
    if True:
        if True:
            for s in range(S):
                # xs2: partitions 0-63 hold x (col c = x[c-3]); partitions
                # 64-127 hold x shifted one column (col c = x[c-2]).
                xs2 = px.tile([128, L + 8], F32R, tag="xs2")
                nc.gpsimd.memset(xs2[0:64, 0:3].bitcast(F32), 0.0)
                nc.gpsimd.memset(xs2[0:64, L + 3:L + 8].bitcast(F32), 0.0)
                nc.gpsimd.memset(xs2[64:128, 0:2].bitcast(F32), 0.0)
                nc.gpsimd.memset(xs2[64:128, L + 2:L + 8].bitcast(F32), 0.0)
                if chunk_load:
                    for ci in range(2):
                        l0 = ci * (L // 2)
                        w = L // 2
                        nc.gpsimd.dma_start(
                            out=xs2[0:64, 3 + l0:3 + l0 + w],
                            in_=x_in[s, :, l0:l0 + w])
                        nc.gpsimd.dma_start(
                            out=xs2[64:128, 2 + l0:2 + l0 + w],
                            in_=xs2[0:64, 3 + l0:3 + l0 + w])
                else:
                    nc.gpsimd.dma_start(out=xs2[0:64, 3:L + 3], in_=x_in[s])
                    nc.gpsimd.dma_start(out=xs2[64:128, 2:L + 2],
                                        in_=xs2[0:64, 3:L + 3])

                # s/d hold the haar sum/diff bands, padded 3 cols each side.
                sfull = psd.tile([128, L2 + 6], F32R, tag="s")
                dfull = psd.tile([128, L2 + 6], F32R, tag="d")
                nc.gpsimd.memset(sfull[:, 0:3].bitcast(F32), 0.0)
                nc.gpsimd.memset(sfull[:, L2 + 3:L2 + 6].bitcast(F32), 0.0)
                nc.gpsimd.memset(dfull[:, 0:3].bitcast(F32), 0.0)
                nc.gpsimd.memset(dfull[:, L2 + 3:L2 + 6].bitcast(F32), 0.0)

                for ci in range(NCH):
                    l0 = ci * CH
                    hp = psum_h.tile([128, CH], F32, tag="hp")
                    for half in range(2):
                        o0 = half * 512
                        for j, k in enumerate(K1BASE):
                            nc.tensor.matmul(
                                hp[:, o0:o0 + 512],
                                wc1[:, j * 128:(j + 1) * 128],
                                xs2[:, l0 + 3 + k + o0:l0 + 3 + k + o0 + 512],
                                start=(j == 0),
                                stop=(j == 3),
                            )
                    # h = gelu(scale * conv1 + bias)  (BN folded in)
                    h = ph.tile([128, CH], F32, tag="h")
                    nc.scalar.activation(h[:], hp[:], gelu_func,
                                         bias=v_bias, scale=v_scale)
                    hl = ph.tile([128, CH], F32, tag="hl")
                    if leaky_on_act:
                        nc.scalar.activation(hl[:], h[:], leaky_func, alpha=0.01)
                    else:
                        nc.vector.scalar_tensor_tensor(
                            hl[:], h[:], 0.01, h[:], OP.mult, OP.max)
                    # haar DWT (scale folded into conv2 weights)
                    t0 = l0 // 2
                    nc.vector.tensor_tensor(
                        sfull[:, 3 + t0:3 + t0 + T],
                        hl[:, 0:CH:2], hl[:, 1:CH:2], OP.add)
                    nc.vector.tensor_tensor(
                        dfull[:, 3 + t0:3 + t0 + T],
                        hl[:, 0:CH:2], hl[:, 1:CH:2], OP.subtract)

                for ci in range(NCH):
                    t0 = ci * T
                    l0 = ci * CH
                    pa = psum_a.tile([128, T], F32, tag="pa")
                    pd = psum_d.tile([128, T], F32, tag="pd")
                    for j in range(7):
                        nc.tensor.matmul(
                            pa[:],
                            wc2[:, j * 128:(j + 1) * 128],
                            sfull[:, t0 + j:t0 + j + T],
                            start=(j == 0), stop=False,
                        )
                    for m in range(3):
                        nc.tensor.matmul(
                            pd[:],
                            wc2[:, (7 + m) * 128:(8 + m) * 128],
                            dfull[:, t0 + 2 + m:t0 + 2 + m + T],
                            start=(m == 0), stop=False,
                        )
                    # skip (1x1 grouped conv) folded into the IDWT psums:
                    # A += (sk_e + sk_o)/2, D += (sk_e - sk_o)/2 via one
                    # stride-2 rhs over the double-copy layout (lower
                    # partitions see x_even, upper see x_odd).
                    rhs_sk = xs2[:, 3 + 2 * t0: 3 + 2 * t0 + 2 * T: 2]
                    nc.tensor.matmul(pa[:], wc2[:, 10 * 128:11 * 128],
                                     rhs_sk, start=False, stop=True)
                    nc.tensor.matmul(pd[:], wc2[:, 11 * 128:12 * 128],
                                     rhs_sk, start=False, stop=True)
                    # drain A to SBUF (HW: only one TT operand may be PSUM)
                    asb = pf.tile([128, T], F32, tag="asb")
                    nc.vector.tensor_copy(asb[:], pa[:])
                    # haar IDWT: interleave (A+D, A-D) = rec + skip
                    f = pf.tile([128, CH], F32, tag="f")
                    nc.vector.tensor_tensor(f[:, 0:CH:2], asb[:], pd[:], OP.add)
                    nc.vector.tensor_tensor(f[:, 1:CH:2], asb[:], pd[:], OP.subtract)
                    # final leaky_relu
                    o = pf.tile([128, CH], F32, tag="o")
                    if leaky_on_act:
                        nc.scalar.activation(o[:], f[:], leaky_func, alpha=0.01)
                    else:
                        nc.vector.scalar_tensor_tensor(
                            o[:], f[:], 0.01, f[:], OP.mult, OP.max)
                    # un-permute channels: partition q -> channel 2q,
                    # partition 64+q -> channel 2q+1
                    nc.sync.dma_start(out=y_out[s, 0:C:2, l0:l0 + CH],
                                      in_=o[0:64, :])
                    nc.sync.dma_start(out=y_out[s, 1:C:2, l0:l0 + CH],
                                      in_=o[64:128, :])


def prepare_weights(w1, g1, b1, m1, v1, w2L, w2H, w_skip):
    """Host-side packing of all weight/BN tensors into the DRAM params."""
    w1 = np.asarray(w1, np.float32).reshape(C, 7)
    w2L = np.asarray(w2L, np.float32).reshape(C, 7)
    w2H = np.asarray(w2H, np.float32).reshape(C, 3)
    w_skip = np.asarray(w_skip, np.float32).reshape(C)
    g1 = np.asarray(g1, np.float32)
    b1 = np.asarray(b1, np.float32)
    m1 = np.asarray(m1, np.float32)
    v1 = np.asarray(v1, np.float32)

    # Channel permutation: SBUF partition p holds output channel phi(p).
    phi = np.array([2 * (p % 64) + p // 64 for p in range(128)])

    # conv1 lhsT per pass: [K=128, M=128]; rows 0-63 tap k, rows 64-127 tap k+1
    w_c1 = np.zeros((4, 128, 128), np.float32)
    for j, k in enumerate(K1BASE):
        for p in range(128):
            c = phi[p]
            i = p % 64  # = c // 2
            w_c1[j, i, p] = w1[c, k + 3]
            if k + 1 <= 3:  # last pass has no second tap
                w_c1[j, 64 + i, p] = w1[c, k + 4]

    # conv2 diagonal lhsT; both 1/sqrt2 haar factors folded in => /2
    w_c2 = np.zeros((12, 128, 128), np.float32)
    for j in range(7):
        np.fill_diagonal(w_c2[j], w2L[phi, j] * 0.5)
    for m in range(3):
        np.fill_diagonal(w_c2[7 + m], w2H[phi, m] * 0.5)
    # skip folded into the IDWT psums: A += (sk_e+sk_o)/2, D += (sk_e-sk_o)/2
    for p in range(128):
        i = p % 64
        w = w_skip[phi[p]] * 0.5
        w_c2[10, i, p] = w        # x_even rows
        w_c2[10, 64 + i, p] = w   # x_odd rows
        w_c2[11, i, p] = w
        w_c2[11, 64 + i, p] = -w

    inv = g1 / np.sqrt(v1 + EPS)
    bias = b1 - m1 * inv
    vecs = np.zeros((128, 4), np.float32)
    vecs[:, 0] = inv[phi]            # ACT scale
    vecs[:, 1] = bias[phi]           # ACT bias
    vecs[:, 2] = w_skip[phi]         # skip scale per out-channel
    return w_c1, w_c2, vecs


_NC_CACHE = {}


def _get_program():
    key = "hw"
    if key not in _NC_CACHE:
        _NC_CACHE[key] = build_program()
    return _NC_CACHE[key]


def kernel(x, w1, g1, b1, m1, v1, w2L, w2H, w_skip):
    x = np.ascontiguousarray(np.asarray(x, np.float32))
    assert x.shape == (NCORES * S, CIN, L), x.shape

    w_c1, w_c2, vecs = prepare_weights(w1, g1, b1, m1, v1, w2L, w2H, w_skip)

    nc = _get_program()
    in_maps = []
    for i in range(NCORES):
        in_maps.append({
            "x": np.ascontiguousarray(x[i * S:(i + 1) * S]),
            "w_c1": w_c1,
            "w_c2": w_c2,
            "vecs": vecs,
        })
    res = run_bass_kernel_spmd(nc, in_maps, list(range(NCORES))).results
    out = np.concatenate([res[i]["y"] for i in range(NCORES)], axis=0)
    return out
